# revision 22
# baseline (speedup 1.0000x reference)
"""Clustered-attention Trainium2 kernel (Bass/Tile), 8-core SPMD.

Problem (per batch b, variable k, with L=512, V=32, D=64, C=8 clusters):
    S      = sum_v key[b,:,v,:]                       # (L, D) shared key-sum
    sc     = query[b,:,k,:] @ S.T / sqrt(D)           # (L, L)
    sc     = where(label[i]==label[j], sc, -inf)
    A      = softmax(sc, axis=-1)
    out    = A @ value[b,:,k,:]

Sharding: 8 cores = 4 batches x 2 halves of the v axis (16 heads/core).

Device algorithm per core (all FLOPs on device):
  - keysum S via tree adds (split across DVE and GpSimd), PE-transposed
    into per-chunk S^T lhsT tiles so heads can start on chunk 0 early.
  - The cluster mask is folded into the scores matmul: the contraction dim
    is extended by 8 one-hot label rows scaled by 8*B (B=96) on the lhsT
    side and 1.0 on the rhs side, so z = q.s + 8B*[same cluster]; the exp
    activation computes exp(z/8 - B), which is exp(q.s/8) for same-cluster
    pairs and <= e^-61 (vs real terms >= e^-35) otherwise -- i.e. an exact
    -inf mask up to ~1e-10 relative.
  - scores^T chunks [128j, 512i] on PE (fp16 operands, fp32 accumulate),
    exp on ScalarE in [128, 1024] pairs (bf16 out), then the A@V matmul
    accumulates E^T chunks as lhsT so the output lands directly as
    [i, d|denom] in PSUM; one reciprocal + broadcast multiplies per head
    normalize it. The softmax denominator comes from a ones-column
    appended to V inside the same matmul.
"""

import numpy as np

import concourse.bass as bass
import concourse.tile as tile
from concourse import mybir
from concourse.masks import make_identity
from concourse.tile import TileContext, ScopedClock

B, L, V, D = 4, 512, 32, 64
NC = 8  # cores
VH = V // 2  # heads (variables) per core
NJ = L // 128  # j/i chunks
BIAS = 96.0  # mask bias (see module docstring)
F32 = mybir.dt.float32
F16 = mybir.dt.float16
BF16 = mybir.dt.bfloat16

PROFILE = False  # set True from a harness to enable NTFF tracing
LAST_RESULT = None  # BassKernelResults of the most recent run

_PATCHED = False


def _patch_tile_drain():
    """Walrus on this image rejects multiple sync-waits on one instruction
    ("Too many sync wait commands"). Legalize by splitting surplus waits
    onto NoOp instructions inserted just before, on the same engine —
    identical semantics (the engine stalls at each wait in order)."""
    global _PATCHED
    if _PATCHED:
        return
    _PATCHED = True

    _orig_add = TileContext._add_instruction

    def _add_instruction(self, inst):
        si = getattr(inst, "sync_info", None)
        if (
            si is not None
            and si.on_wait
            and len(si.on_wait) > 1
            and inst.engine != mybir.EngineType.Unassigned
        ):
            waits = list(si.on_wait)
            for w in waits[:-1]:
                nop = mybir.InstNoOp(name=self.nc.get_next_instruction_name())
                nop.engine = inst.engine
                nop.sync_info = mybir.SyncInfo(on_wait=[w], on_update=[])
                _orig_add(self, nop)
            inst.sync_info = mybir.SyncInfo(
                on_wait=[waits[-1]], on_update=list(si.on_update or [])
            )
        _orig_add(self, inst)

    TileContext._add_instruction = _add_instruction

    def _drain_and_barrier(self, tick_clock, wait_clock):
        nc = self.nc
        drain_inst = nc.sync.drain()
        wait_clock.add_sem_waits(
            drain_inst.ins, ScopedClock({None: tick_clock.global_clock})
        )
        si = drain_inst.ins.sync_info
        if si is not None and si.on_wait and len(si.on_wait) > 1:
            waits = list(si.on_wait)
            drain_inst.ins.sync_info = mybir.SyncInfo(
                on_wait=waits[:1], on_update=list(si.on_update or [])
            )
            for i in range(1, len(waits)):
                nop = nc.sync.nop(nofuse=True, hint=f"drain_split_{i}")
                nop.ins.sync_info = mybir.SyncInfo(on_wait=[waits[i]], on_update=[])
        nc.all_engine_barrier()
        assert self.sems is not None
        popped = nc._tile_sem_poison_stack.pop()
        assert popped is self._sem_poison
        nc.clear_and_free_semaphores(list(self.sems.allocated().values()))
        nc.all_engine_barrier()

    TileContext._drain_and_barrier = _drain_and_barrier


def _tree_reduce_v(eng, pool, kc, tag):
    """Sum kc [128, V*D] over the v axis -> [128, D] via contiguous
    halving adds on the given engine (v-major layout: halves contiguous)."""
    cur = kc
    width = V * D
    while width > D:
        width //= 2
        nxt = pool.tile([128, width], kc.dtype, tag=f"red_{width}")
        eng.tensor_tensor(
            out=nxt, in0=cur[:, 0:width], in1=cur[:, width : 2 * width],
            op=mybir.AluOpType.add,
        )
        cur = nxt
    return cur


def _dma_split(nc, out_ap, in_ap, parts):
    """Issue `parts` dma_starts over free-dim slices so the transfer
    spreads across DMA queues instead of serializing on one."""
    w = out_ap.shape[-1]
    step = w // parts
    for i in range(parts):
        sl = slice(i * step, (i + 1) * step) if i < parts - 1 else slice(i * step, w)
        nc.sync.dma_start(out=out_ap[..., sl], in_=in_ap[..., sl])


def _build_nc():
    nc = bass.Bass("TRN2", target_bir_lowering=False, debug=False)

    # 16-bit inputs, prepared host-side in kernel(): q_t is the fp16
    # pre-transposed query shard (D, VH*L), k16 the fp16 key, vp the bf16
    # value with a ones column appended (in-matmul softmax denominator)
    q_t = nc.dram_tensor("q_t", [D, VH * L], F16, kind="ExternalInput").ap()
    k_in = nc.dram_tensor("k", [L, V, D], F16, kind="ExternalInput").ap()
    v_in = nc.dram_tensor("v", [L, VH, D + 2], BF16, kind="ExternalInput").ap()
    lab = nc.dram_tensor("lab", [1, L], F32, kind="ExternalInput").ap()
    iota8 = nc.dram_tensor("iota8", [8, 1], F32, kind="ExternalInput").ap()
    o_out = nc.dram_tensor("o", [L, VH, D], F32, kind="ExternalOutput").ap()

    with TileContext(nc) as tc:
        with (
            tc.tile_pool(name="singles", bufs=1) as singles,
            tc.tile_pool(name="redpool", bufs=2) as redpool,
            tc.tile_pool(name="epool", bufs=6) as epool,
            tc.tile_pool(name="rpool", bufs=3) as rpool,
            tc.tile_pool(name="ps_score", bufs=2, space="PSUM") as ps_score,
            tc.tile_pool(name="ps_u", bufs=2, space="PSUM") as ps_u,
            tc.tile_pool(name="ps_t", bufs=1, space="PSUM") as ps_t,
        ):
            # ---- tiny control inputs first on the GpSimd (SWDGE) issue
            # path, which starts ~5us earlier than SP's ----
            lab_sb = singles.tile([8, L], F32)
            lab_bcast = bass.AP(tensor=lab.tensor, offset=lab.offset,
                                ap=[[0, 8]] + list(lab.ap[1:]))
            nc.gpsimd.dma_start(out=lab_sb, in_=lab_bcast)
            iota_sb = singles.tile([8, 1], F32)
            nc.gpsimd.dma_start(out=iota_sb, in_=iota8)

            # ---- bulk input DMAs. Issue rate (~0.6us per dma_start on a
            # sequencer) dominates the prologue, so the count is kept low
            # and the earliest-needed tensors are issued first. ----
            kcs = []
            for jc in range(NJ):
                kcs.append(singles.tile([128, V * D], F16,
                                        tag=f"kc{jc}", name=f"kc{jc}"))
            for jc in range(2):
                for p in range(2):
                    sl = slice(p * 1024, (p + 1) * 1024)
                    nc.gpsimd.dma_start(
                        out=kcs[jc][:, sl],
                        in_=k_in[jc * 128 : (jc + 1) * 128]
                        .rearrange("p v d -> p (v d)")[:, sl],
                    )
            # 4 query groups of 4 heads, fp16, straight into lhsT layout
            qtbg = [
                singles.tile([D + 8, 4 * L], F16, tag=f"qtbg{g}", name=f"qtbg{g}")
                for g in range(4)
            ]
            for p in range(2):
                sl = slice(p * 1024, (p + 1) * 1024)
                nc.gpsimd.dma_start(out=qtbg[0][0:D, sl], in_=q_t[:, sl])
            for jc in range(2, NJ):
                for p in range(2):
                    sl = slice(p * 1024, (p + 1) * 1024)
                    nc.gpsimd.dma_start(
                        out=kcs[jc][:, sl],
                        in_=k_in[jc * 128 : (jc + 1) * 128]
                        .rearrange("p v d -> p (v d)")[:, sl],
                    )
            for g in range(1, 4):
                for p in range(2):
                    sl = slice(p * 1024, (p + 1) * 1024)
                    nc.gpsimd.dma_start(
                        out=qtbg[g][0:D, sl],
                        in_=q_t[:, g * 4 * L + p * 1024 :
                                g * 4 * L + (p + 1) * 1024],
                    )
            # value chunks (bf16, ones column pre-padded by the host)
            vcast = singles.tile([128, NJ, VH, D + 2], BF16)
            for jc in range(NJ):
                nc.gpsimd.dma_start(
                    out=vcast[:, jc],
                    in_=v_in[jc * 128 : (jc + 1) * 128],
                )

            # ---- constants ----
            identity = singles.tile([128, 128], F16)
            make_identity(nc, identity)
            negb = singles.tile([128, 1], F32)
            nc.vector.memset(negb, -BIAS)
            # preload the exp activation table (~1.3us) during DMA wait
            dummy = singles.tile([128, 1], F32)
            nc.scalar.activation(dummy, negb,
                                 mybir.ActivationFunctionType.Exp)
            junk = singles.tile([128, L], F16)
            nc.vector.memset(junk, 1.0)

            # one-hot label rows (device-computed from labels)
            onehot = singles.tile([8, L], F32)
            nc.vector.tensor_scalar(onehot, lab_sb, iota_sb, None,
                                    op0=mybir.AluOpType.is_equal)
            oh16 = singles.tile([8, L], F16)
            nc.vector.tensor_copy(oh16, onehot)
            oh768 = singles.tile([8, L], F16)
            nc.vector.tensor_scalar_mul(oh768, onehot, 8.0 * BIAS)
            # replicate the one-hot rows into each query group (one
            # 0-stride SBUF->SBUF DMA per group, issued on SP)
            for g in range(4):
                oh_rep = bass.AP(tensor=oh16.tensor, offset=oh16.offset,
                                 ap=[list(oh16.ap[0]), [0, 4],
                                     list(oh16.ap[1])])
                nc.sync.dma_start(
                    out=qtbg[g][D : D + 8, :].rearrange(
                        "p (h l) -> p h l", h=4, l=L),
                    in_=oh_rep,
                )

            # ---- PE warmup on junk data so HAM reaches 2.4 GHz ----
            for w in range(4):
                wps = ps_u.tile([128, L], F32, tag="warm", name=f"warm{w}", bufs=1)
                nc.tensor.matmul(wps, lhsT=identity, rhs=junk,
                                 start=True, stop=True)

            # ---- keysum -> per-chunk S^T lhsT tiles (fp16 tree adds) ----
            stbs = {}

            def _make_stb(jc):
                s_chunk = _tree_reduce_v(nc.vector, redpool, kcs[jc], f"red{jc}")
                st_ps = ps_t.tile([D, 128], F16, tag="st_ps", name=f"st{jc}")
                nc.tensor.transpose(st_ps, s_chunk, identity)
                stb = singles.tile([D + 8, 128], F16, tag=f"stb{jc}",
                                   name=f"stb{jc}")
                nc.vector.tensor_copy(stb[0:D, :], st_ps)
                nc.sync.dma_start(out=stb[D : D + 8, :],
                                  in_=oh768[:, jc * 128 : (jc + 1) * 128])
                stbs[jc] = stb

            HG = VH // 4
            oc_tiles = [
                singles.tile([128, NJ, HG, D], F32, tag=f"oc{g}", name=f"oc{g}")
                for g in range(4)
            ]

            def _head_pair(h, jcp):
                qtb = qtbg[h // 4]
                hh = h % 4
                ps = ps_score.tile([128, 2, L], F32, tag="ps",
                                   name=f"ps{h}_{jcp}")
                for t in range(2):
                    nc.tensor.matmul(
                        ps[:, t, :], lhsT=stbs[2 * jcp + t],
                        rhs=qtb[:, hh * L : (hh + 1) * L],
                        start=True, stop=True,
                    )
                e_t = epool.tile([128, 2, L], BF16, tag="et",
                                 name=f"et{h}_{jcp}")
                nc.scalar.activation(
                    e_t, ps, mybir.ActivationFunctionType.Exp,
                    bias=negb, scale=1.0 / 8.0,
                )
                return e_t

            def _head_tail(h, e_tiles):
                oc = oc_tiles[h // HG]
                hh = h % HG
                psu = ps_u.tile([128, NJ, D + 1], F32, tag="psu", name=f"psu{h}")
                for si in range(NJ):
                    for jc in range(NJ):
                        nc.tensor.matmul(
                            psu[:, si, :],
                            lhsT=e_tiles[jc // 2][:, jc % 2,
                                                  si * 128 : (si + 1) * 128],
                            rhs=vcast[:, jc, h, 0 : D + 1],
                            start=(jc == 0), stop=(jc == NJ - 1),
                        )
                rinv = rpool.tile([128, NJ], F32, tag="rinv", name=f"rinv{h}")
                nc.vector.reciprocal(rinv, psu[:, :, D])
                rinv_b = bass.AP(
                    tensor=rinv.tensor, offset=rinv.offset,
                    ap=[list(rinv.ap[0]), list(rinv.ap[1]), [0, D]],
                )
                nc.vector.tensor_tensor(
                    out=oc[:, :, hh, :], in0=psu[:, :, 0:D], in1=rinv_b,
                    op=mybir.AluOpType.mult,
                )
                if hh == HG - 1:
                    g = h // HG
                    for si in range(NJ):
                        nc.sync.dma_start(
                            out=o_out[si * 128 : (si + 1) * 128,
                                      g * HG : (g + 1) * HG, :],
                            in_=oc[:, si],
                        )

            # ---- interleave: head-0 pair0 right after stb0/stb1 so the
            # in-order PE stream matches data arrival; A@V runs one head
            # behind the scores so the PE never stalls on the live exp ----
            _make_stb(0)
            _make_stb(1)
            e_first = [_head_pair(0, 0)]
            _make_stb(2)
            _make_stb(3)
            e_first.append(_head_pair(0, 1))
            prev = (0, e_first)
            for h in range(1, VH):
                e_tiles = [_head_pair(h, 0), _head_pair(h, 1)]
                _head_tail(*prev)
                prev = (h, e_tiles)
            _head_tail(*prev)
    return nc


_NC_CACHE = None


def _get_nc():
    global _NC_CACHE
    if _NC_CACHE is None:
        _patch_tile_drain()
        _NC_CACHE = _build_nc()
    return _NC_CACHE


def kernel(query, key, value, label_arr):
    """Full inputs (B,L,V,D)/(B,L) -> full output (B,L,V,D)."""
    global LAST_RESULT
    import ml_dtypes
    from concourse.bass_utils import run_bass_kernel_spmd

    query = np.asarray(query, dtype=np.float32)
    key = np.asarray(key, dtype=np.float32)
    value = np.asarray(value, dtype=np.float32)
    lab_f32 = np.asarray(label_arr).astype(np.float32)
    iota = np.arange(8, dtype=np.float32).reshape(8, 1)

    in_maps = []
    for c in range(NC):
        b, v0 = c // 2, (c % 2) * VH
        vp = np.zeros((L, VH, D + 2), dtype=ml_dtypes.bfloat16)
        vp[:, :, 0:D] = value[b, :, v0 : v0 + VH, :].astype(ml_dtypes.bfloat16)
        vp[:, :, D] = 1.0
        in_maps.append({
            "q_t": np.ascontiguousarray(
                query[b, :, v0 : v0 + VH, :].transpose(2, 1, 0)
            ).astype(np.float16).reshape(D, VH * L),
            "k": key[b].astype(np.float16),
            "v": vp,
            "lab": lab_f32[b].reshape(1, L).copy(),
            "iota8": iota,
        })

    nc = _get_nc()
    kwargs = {}
    if PROFILE:
        kwargs["trace"] = True
    res = run_bass_kernel_spmd(nc, in_maps, list(range(NC)), **kwargs)
    LAST_RESULT = res

    out = np.empty((B, L, V, D), dtype=np.float32)
    for c in range(NC):
        b, v0 = c // 2, (c % 2) * VH
        out[b, :, v0 : v0 + VH, :] = res.results[c]["o"]
    return out
